# revision 1
# baseline (speedup 1.0000x reference)
"""Trainium2 Bass kernel for the spectral ConvolutionLayer problem.

Math: with u (B=2, L=4096, D=256), eigenvectors ev (K=16, L), eigenvalues
lam (K,), M (K, 256, 256):

    conv[b,k,d,l] = sum_t u[b,t,d] * ev[k, (l-t) mod L]       (circular conv)
    out[b,l,p]    = sum_{k,d} conv[b,k,d,l] * lam[k] * M[k,d,p]

Equivalently out[b] = sum_k (C_k @ u[b]) @ (lam_k M_k) with the circulant
C_k[l,t] = ev[k,(l-t) mod L].

Sharding: output rows l are sharded across 8 cores (512 rows each) — no
collectives.  Per core and filter k the needed circulant slice is a Hankel
matrix in disguise: after reversing the partition order of both matmul
operands, its tiles become plain overlapping-window reads
C_buf[q, col] = ev_ext[q + col] from a small host-prepared extended buffer
ev_ext[i] = ev[k, (l_off + i - (L-1)) mod L].  One 1.1 MB DMA per filter
materializes all circulant tiles for that filter.

Per-core pipeline (all matmuls bf16, fp32 PSUM accumulate):
  stage 1: y[b,k][d,l'] = sum_jr u_rev_tile(jr)^T @ C_buf[:, 128jr:128jr+512]
  stage 2: outT[b][p,l'] += (lam_k M_k)[d,p]^T-side matmul with rhs y
Output is written transposed (B, D, 512) and fixed up on host.
"""

import numpy as np
import ml_dtypes

import concourse.bacc as bacc
import concourse.bass as bass
import concourse.mybir as mybir
import concourse.tile as tile
from concourse.bass_utils import run_bass_kernel_spmd

B, L, D, K = 2, 4096, 256, 16
NCORES = 8
LSH = L // NCORES          # 512 output rows per core
NJR = L // 128             # 32 contraction tiles
EXT = 4608                 # extended eigenvector buffer length (>= 128*31+512+127+1)
CB_W = 128 * (NJR - 1) + LSH   # 4480 C-buffer width
NREP_EV = 8                # HBM replicas of ev_ext to spread DMA hot-spot
BF16 = mybir.dt.bfloat16
F32 = mybir.dt.float32
NPBF16 = ml_dtypes.bfloat16

_CACHE = {}


def _build_bass():
    nc = bacc.Bacc("TRN2", target_bir_lowering=False)
    u_h = nc.dram_tensor("u_rev", [B, L, D], BF16, kind="ExternalInput")
    m_h = nc.dram_tensor("m_mat", [K, 2, 128, D], BF16, kind="ExternalInput")
    # 8 identical replicas of the extended eigenvector buffer.  The C-buffer
    # expansion reads ~18 MB through overlapping windows over a ~9 KB
    # footprint per filter; replicas spread concurrent SDMA reads across 8x
    # more HBM pages to avoid bank hot-spotting.
    e_h = nc.dram_tensor("ev_ext", [NREP_EV, K, EXT], BF16, kind="ExternalInput")
    o_h = nc.dram_tensor("out_t", [B, D, LSH], F32, kind="ExternalOutput")

    with tile.TileContext(nc) as tc:
        with (
            tc.tile_pool(name="const", bufs=1) as const_pool,
            tc.tile_pool(name="cbufp", bufs=2) as cbuf_pool,
            tc.tile_pool(name="ysbp", bufs=8) as y_pool,
            tc.tile_pool(name="osbp", bufs=2) as o_pool,
            tc.tile_pool(name="psyp", bufs=2, space="PSUM") as psy_pool,
            tc.tile_pool(name="psop", bufs=4, space="PSUM") as pso_pool,
        ):
            # m_sb[di, k, dh, p] = m_mat[k, dh, di, p]  (= lam_k*M[k, 128dh+di, p])
            m_sb = const_pool.tile([128, K, 2, D], BF16, name="m_sb")
            nc.sync.dma_start(out=m_sb, in_=m_h[:].rearrange("k h i p -> i k h p"))
            # u_sb[ti, b, jr, d] = u_rev[b, 128*jr + ti, d].  Split into 8
            # slice DMAs (issued in consumption order) so the first stage-1
            # matmul group only waits on its 528 KB slice, not the full
            # 4.2 MB; the rest streams in under compute.
            u_sb = const_pool.tile([128, B, NJR, D], BF16, name="u_sb")
            JQ = NJR // 4
            for b in range(B):
                for jq in range(4):
                    nc.sync.dma_start(
                        out=u_sb[:, b, JQ * jq:JQ * (jq + 1), :],
                        in_=u_h[b, 128 * JQ * jq:128 * JQ * (jq + 1), :]
                        .rearrange("(jr p) d -> p jr d", p=128),
                    )

            # Warm the PE pstate ramp while the u/cb DMAs are in flight:
            # dummy matmuls on m_sb (tiny DMA, lands first) keep the PE busy
            # so the main loop enters at full clock instead of ramping.
            with tc.tile_pool(name="warmp", bufs=1, space="PSUM") as warm_pool:
                warm_ps = warm_pool.tile([128, 2 * D], F32, name="warm_ps")
                for w in range(5):
                    nc.tensor.matmul(
                        warm_ps,
                        m_sb[:, 0, 0, 0:128],
                        m_sb[:, w, :, :],
                        start=(w == 0),
                        stop=(w == 4),
                    )

            # output accumulators: outT[b][ph][p_i, l'] , p = 128*ph + p_i
            out_ps = {}
            for b in range(B):
                for ph in range(2):
                    out_ps[(b, ph)] = pso_pool.tile(
                        [128, LSH], F32, name=f"out_ps_{b}_{ph}",
                        tag=f"out_ps_{b}_{ph}", bufs=1,
                    )

            for k in range(K):
                # C_buf[q, col] = ev_ext[k, q + col] : overlapping-window DMA
                # C_buf[q, col] = ev_ext[k, q + col] : overlapping-window DMAs,
                # one 16-partition group per eigenvector replica.
                cb = cbuf_pool.tile([128, CB_W], BF16, name="cb", tag="cb")
                rows = 128 // NREP_EV
                for i in range(NREP_EV):
                    nc.gpsimd.dma_start(
                        out=cb[rows * i:rows * (i + 1), :],
                        in_=bass.AP(
                            e_h,
                            (i * K + k) * EXT + rows * i,
                            [[1, rows], [1, CB_W]],
                        ),
                    )
                for b in range(B):
                    for dh in range(2):
                        psy = psy_pool.tile([128, LSH], F32, name="psy", tag="psy")
                        for jr in range(NJR):
                            nc.tensor.matmul(
                                psy,
                                u_sb[:, b, jr, dh * 128:(dh + 1) * 128],
                                cb[:, 128 * jr:128 * jr + LSH],
                                start=(jr == 0),
                                stop=(jr == NJR - 1),
                            )
                        ysb = y_pool.tile([128, LSH], BF16, name="ysb", tag="ysb")
                        nc.vector.tensor_copy(ysb, psy)
                        for ph in range(2):
                            nc.tensor.matmul(
                                out_ps[(b, ph)],
                                m_sb[:, k, dh, ph * 128:(ph + 1) * 128],
                                ysb,
                                start=(k == 0 and dh == 0),
                                stop=(k == K - 1 and dh == 1),
                            )

            for b in range(B):
                for ph in range(2):
                    osb = o_pool.tile([128, LSH], F32, name="osb", tag="osb")
                    nc.vector.tensor_copy(osb, out_ps[(b, ph)])
                    nc.sync.dma_start(
                        out=o_h[b, ph * 128:(ph + 1) * 128, :], in_=osb
                    )
    nc.finalize()
    return nc


def _prep_inputs(u, eigenvectors, eigenvalues, M):
    u = np.asarray(u, dtype=np.float32)
    ev = np.asarray(eigenvectors, dtype=np.float32)
    lam = np.asarray(eigenvalues, dtype=np.float32)
    M = np.asarray(M, dtype=np.float32)

    u_rev = np.ascontiguousarray(u[:, ::-1, :]).astype(NPBF16)
    m_mat = np.ascontiguousarray(
        (lam[:, None, None] * M).astype(NPBF16).reshape(K, 2, 128, D)
    )
    in_maps = []
    idx = np.arange(EXT)
    for c in range(NCORES):
        l_off = LSH * c
        ev_ext = ev[:, (l_off + idx - (L - 1)) % L].astype(NPBF16)
        ev_rep = np.ascontiguousarray(
            np.broadcast_to(ev_ext[None], (NREP_EV, K, EXT))
        )
        in_maps.append({"u_rev": u_rev, "m_mat": m_mat, "ev_ext": ev_rep})
    return in_maps


def _run(inputs, trace=False):
    if "nc" not in _CACHE:
        _CACHE["nc"] = _build_bass()
    nc = _CACHE["nc"]
    in_maps = _prep_inputs(**inputs)
    res = run_bass_kernel_spmd(
        nc, in_maps, core_ids=list(range(NCORES)), trace=trace
    )
    out = np.empty((B, L, D), dtype=np.float32)
    for c in range(NCORES):
        out[:, LSH * c:LSH * (c + 1), :] = np.asarray(
            res.results[c]["out_t"]
        ).transpose(0, 2, 1)
    return out, res


def kernel(**inputs):
    out, _ = _run(inputs, trace=False)
    return out



# revision 10
# speedup vs baseline: 1.5319x; 1.5319x over previous
"""Trainium2 Bass kernel for the spectral ConvolutionLayer problem.

Math: with u (B=2, L=4096, D=256), eigenvectors ev (K=16, L), eigenvalues
lam (K,), M (K, 256, 256):

    conv[b,k,d,l] = sum_t u[b,t,d] * ev[k, (l-t) mod L]       (circular conv)
    out[b,l,p]    = sum_{k,d} conv[b,k,d,l] * lam[k] * M[k,d,p]

Radix-2 split of the length-L circular convolution (H = L/2):
    u+/- = u[0:H] +/- u[H:L],  e+/- = ev[0:H] +/- ev[H:L]
    A = u+ (*)_cyclic(H) e+          N = u- (*)_negacyclic(H) e-
    y[s] = (A[s]+N[s])/2,  y[s+H] = (A[s]-N[s])/2
This halves the matmul row count (contraction L -> H for two half-size
convs whose outputs are shared by the two output halves).  Both half
convs are Toeplitz, so the same overlapping-window Hankel trick applies:
after reversing the partition order of the u operand, tiles of the conv
matrix are plain overlapping-window reads C[q, col] = ext[q + col] from a
small host-prepared extended buffer (negacyclic sign flips baked in).

Sharding: each core owns 256 output rows s in [256c, 256c+256) of BOTH
half convs, producing output rows l = s and l = s + 2048 — no
collectives.  Per-core pipeline (matmuls bf16, fp32 PSUM accumulate):
  stage 1: psyC/psyN[d, s'] = sum_jr u+/-_rev_tile(jr)^T @ C_cyc/nega
  butterfly: ysb = [psyC+psyN | psyC-psyN]  (the 1/2 is folded into M)
  stage 2: outT[b][p, :] += (0.5 lam_k M_k)^T-side matmul with rhs ysb
Output is written transposed (B, D, 512) and fixed up on host.
"""

import numpy as np
import ml_dtypes

import concourse.bacc as bacc
import concourse.bass as bass
import concourse.mybir as mybir
import concourse.tile as tile
from concourse.bass_utils import run_bass_kernel_spmd

B, L, D, K = 2, 4096, 256, 16
H = L // 2                 # 2048 half-conv length
NCORES = 8
SSH = H // NCORES          # 256 owned half-conv output rows per core
NJR = H // 128             # 16 contraction tiles per half conv
CB_W = 128 * (NJR - 1) + SSH   # 2176 C-buffer width
EXT = 2304                 # extended buffer length (>= 128*15 + 256 + 127 + 1)
NREP_EV = 8                # HBM replicas of ev_ext to spread DMA hot-spot
BF16 = mybir.dt.bfloat16
F32 = mybir.dt.float32
NPBF16 = ml_dtypes.bfloat16

_CACHE = {}


def _build_bass():
    nc = bacc.Bacc("TRN2", target_bir_lowering=False)
    # u_rev[b, half, r, d] = (u+/-)[b, H-1-r, d]
    u_h = nc.dram_tensor("u_rev", [B, 2, H, D], BF16, kind="ExternalInput")
    # m_mat[s] = +/- 0.5*lam*M : the butterfly (A +/- N) is folded into
    # stage 2 as two matmuls per output half, the minus via the negated copy.
    m_h = nc.dram_tensor("m_mat", [2, K, 2, 128, D], BF16, kind="ExternalInput")
    # 8 identical replicas of the two extended eigenvector buffers
    # (cyc/nega).  The C-buffer expansion reads ~18 MB through overlapping
    # windows over a ~9 KB footprint per filter; replicas spread concurrent
    # SDMA reads across 8x more HBM pages to avoid bank hot-spotting.
    e_h = nc.dram_tensor("ev_ext", [NREP_EV, K, 2, EXT], BF16, kind="ExternalInput")
    o_h = nc.dram_tensor("out_t", [B, D, 2 * SSH], F32, kind="ExternalOutput")

    with tile.TileContext(nc) as tc:
        with (
            tc.tile_pool(name="const", bufs=1) as const_pool,
            tc.tile_pool(name="cbufp", bufs=2) as cbuf_pool,
            tc.tile_pool(name="ysbp", bufs=8) as y_pool,
            tc.tile_pool(name="osbp", bufs=2) as o_pool,
            tc.tile_pool(name="psyp", bufs=2, space="PSUM") as psy_pool,
            tc.tile_pool(name="psop", bufs=4, space="PSUM") as pso_pool,
        ):
            # m_sb[di, s, k, dh, p] = m_mat[s, k, dh, di, p]
            m_sb = const_pool.tile([128, 2, K, 2, D], BF16, name="m_sb")
            nc.sync.dma_start(
                out=m_sb, in_=m_h[:].rearrange("s k h i p -> i s k h p")
            )
            # u_sb[ti, b, half, jr, d] = u_rev[b, half, 128*jr + ti, d].  Split
            # into 8 slice DMAs (issued in consumption order) so the first
            # stage-1 matmul group only waits on its slice, not the full 4 MB.
            u_sb = const_pool.tile([128, B, 2, NJR, D], BF16, name="u_sb")
            JQ = NJR // 2
            for b in range(B):
                for half in range(2):
                    for jq in range(2):
                        nc.sync.dma_start(
                            out=u_sb[:, b, half, JQ * jq:JQ * (jq + 1), :],
                            in_=u_h[b, half, 128 * JQ * jq:128 * JQ * (jq + 1), :]
                            .rearrange("(jr p) d -> p jr d", p=128),
                        )

            # Warm the PE pstate ramp while the u/cb DMAs are in flight.
            with tc.tile_pool(name="warmp", bufs=1, space="PSUM") as warm_pool:
                warm_ps = warm_pool.tile([128, 2 * D], F32, name="warm_ps")
                for w in range(5):
                    nc.tensor.matmul(
                        warm_ps,
                        m_sb[:, 0, 0, 0, 0:128],
                        m_sb[:, 0, w, :, :],
                        start=(w == 0),
                        stop=(w == 4),
                    )

            # output accumulators: outT[b][ph][p_i, :] , p = 128*ph + p_i
            out_ps = {}
            for b in range(B):
                for ph in range(2):
                    out_ps[(b, ph)] = pso_pool.tile(
                        [128, 2 * SSH], F32, name=f"out_ps_{b}_{ph}",
                        tag=f"out_ps_{b}_{ph}", bufs=1,
                    )

            for k in range(K):
                # C[q, half, col] = ev_ext[k, half, q + col] : overlapping-
                # window DMAs, one 16-partition group per replica per half.
                cb = cbuf_pool.tile([128, 2, CB_W], BF16, name="cb", tag="cb")
                rows = 128 // NREP_EV
                for i in range(NREP_EV):
                    for half in range(2):
                        nc.gpsimd.dma_start(
                            out=cb[rows * i:rows * (i + 1), half, :],
                            in_=bass.AP(
                                e_h,
                                ((i * K + k) * 2 + half) * EXT + rows * i,
                                [[1, rows], [1, CB_W]],
                            ),
                        )
                for b in range(B):
                    for dh in range(2):
                        psy = psy_pool.tile([128, 2 * SSH], F32, name="psy", tag="psy")
                        psyC = psy[:, 0:SSH]
                        psyN = psy[:, SSH:2 * SSH]
                        # start=True clears has_written for the WHOLE psum
                        # bank, so: one start (very first matmul into the
                        # bank) and one stop (very last).  The first write
                        # to the nega region overwrites stale data because
                        # its has_written bits are clear.
                        for half, psyh in ((0, psyC), (1, psyN)):
                            for jr in range(NJR):
                                nc.tensor.matmul(
                                    psyh,
                                    u_sb[:, b, half, jr, dh * 128:(dh + 1) * 128],
                                    cb[:, half, 128 * jr:128 * jr + SSH],
                                    start=(half == 0 and jr == 0),
                                    stop=(half == 1 and jr == NJR - 1),
                                )
                        # ysb = [A | N] in bf16; the (A +/- N) butterfly is
                        # folded into stage 2 via the +/-m copies.
                        ysb = y_pool.tile([128, 2 * SSH], BF16, name="ysb", tag="ysb")
                        nc.vector.tensor_copy(ysb, psy)
                        first = k == 0 and dh == 0
                        last = k == K - 1 and dh == 1
                        for ph in range(2):
                            ops = out_ps[(b, ph)]
                            mP = m_sb[:, 0, k, dh, ph * 128:(ph + 1) * 128]
                            mN = m_sb[:, 1, k, dh, ph * 128:(ph + 1) * 128]
                            # out[:, 0:S]  += m @ A + m @ N
                            # out[:, S:2S] += m @ A - m @ N
                            # one start / one stop per psum bank (see above)
                            nc.tensor.matmul(
                                ops[:, 0:SSH], mP, ysb[:, 0:SSH],
                                start=first, stop=False,
                            )
                            nc.tensor.matmul(
                                ops[:, 0:SSH], mP, ysb[:, SSH:2 * SSH],
                                start=False, stop=False,
                            )
                            nc.tensor.matmul(
                                ops[:, SSH:2 * SSH], mP, ysb[:, 0:SSH],
                                start=False, stop=False,
                            )
                            nc.tensor.matmul(
                                ops[:, SSH:2 * SSH], mN, ysb[:, SSH:2 * SSH],
                                start=False, stop=last,
                            )

            for b in range(B):
                for ph in range(2):
                    osb = o_pool.tile([128, 2 * SSH], F32, name="osb", tag="osb")
                    nc.vector.tensor_copy(osb, out_ps[(b, ph)])
                    nc.sync.dma_start(
                        out=o_h[b, ph * 128:(ph + 1) * 128, :], in_=osb
                    )
    nc.finalize()
    return nc


def _prep_inputs(u, eigenvectors, eigenvalues, M):
    u = np.asarray(u, dtype=np.float32)
    ev = np.asarray(eigenvectors, dtype=np.float32)
    lam = np.asarray(eigenvalues, dtype=np.float32)
    M = np.asarray(M, dtype=np.float32)

    up = u[:, :H, :] + u[:, H:, :]
    um = u[:, :H, :] - u[:, H:, :]
    u_rev = np.ascontiguousarray(
        np.stack([up[:, ::-1, :], um[:, ::-1, :]], axis=1)
    ).astype(NPBF16)
    # fold the 1/2 butterfly normalization into M; stack +/- copies
    mm = (0.5 * lam[:, None, None] * M).reshape(K, 2, 128, D)
    m_mat = np.ascontiguousarray(np.stack([mm, -mm], axis=0).astype(NPBF16))
    ep = ev[:, :H] + ev[:, H:]
    em = ev[:, :H] - ev[:, H:]
    in_maps = []
    idx = np.arange(EXT)
    for c in range(NCORES):
        delta = SSH * c + idx - (H - 1)          # (EXT,)
        cyc = ep[:, delta % H]                   # (K, EXT)
        sign = np.where((delta // H) % 2 == 0, 1.0, -1.0).astype(np.float32)
        nega = em[:, delta % H] * sign[None, :]
        ev_ext = np.stack([cyc, nega], axis=1).astype(NPBF16)   # (K, 2, EXT)
        ev_rep = np.ascontiguousarray(
            np.broadcast_to(ev_ext[None], (NREP_EV, K, 2, EXT))
        )
        in_maps.append({"u_rev": u_rev, "m_mat": m_mat, "ev_ext": ev_rep})
    return in_maps


def _run(inputs, trace=False):
    if "nc" not in _CACHE:
        _CACHE["nc"] = _build_bass()
    nc = _CACHE["nc"]
    in_maps = _prep_inputs(**inputs)
    res = run_bass_kernel_spmd(
        nc, in_maps, core_ids=list(range(NCORES)), trace=trace
    )
    out = np.empty((B, L, D), dtype=np.float32)
    for c in range(NCORES):
        ot = np.asarray(res.results[c]["out_t"])       # (B, D, 512)
        out[:, SSH * c:SSH * (c + 1), :] = ot[:, :, :SSH].transpose(0, 2, 1)
        out[:, H + SSH * c:H + SSH * (c + 1), :] = (
            ot[:, :, SSH:].transpose(0, 2, 1)
        )
    return out, res


def kernel(**inputs):
    out, _ = _run(inputs, trace=False)
    return out
